# revision 2
# baseline (speedup 1.0000x reference)
"""Trainium2 Bass kernel v4 for nn_CapsuleLayer: 2x2 conv (128->1024) + routing.

Design (4 samples/core on 8 cores, data-parallel over batch):
  - Conv weights-stationary bf16: stationary wtA[c, kpos, blk, 128oc], moving
    x windows -> u_A[oc, n] in PSUM. Epilogue = ACT Identity copy with
    per-partition bias (conv bias folded here: ua = u0 + beta) and accum_out
    riding to produce o0 row-sums.
  - n = 4096 raw positions (32 chunks x 128), pos p = 64h + w; fakes at
    p%64==63, junk at p>=4032: zeroed via rm8 routing mask; o0 corrected by
    subtracting fake/junk column sums (f1/f2).
  - Transposes (PE, bf16) interleave with conv tile-groups so the DVE
    PSUM->SBUF copies (ptr -> ut fp8) hide under conv matmul streams.
  - b-mm flipped: obdt [128oc, 32i] bf16 stationary, streams ua ->
    pb [32i, 1024n] f32; ACT exp directly on PSUM (bias already in ua).
  - softmax: PE-transpose exp -> eT [n, i] bf16 PSUM, DVE z/recip; ct fp8
    scaled x8 (rm8 = 8, maskdiag = 1/8) to avoid fp8 subnormals.
  - o-mm: fp8 DoubleRow, ct2 [128, 2, 32] stationary, ut2 [128, 2, 256]
    moving, K=256 chunk pairs -> po [32 i, 1024 oc]; diag extraction on DVE.
  - PSUM: tagA [128,1024]f32 x2 (conv pc + pb + irep share), pmix
    [128,8,128]bf16 x2 (ptr + eT + oT), po [32,1024]f32 x1 = 8 banks.
"""
import os
import sys

sys.path.insert(0, "/opt/trn_rl_repo")

import numpy as np
import ml_dtypes

B, IN_C, H, W_SP = 32, 128, 64, 64
NUM_CAPS, D0, D1 = 32, 4, 8
OUT_C = NUM_CAPS * D0 * D1  # 1024
N_CORES = 8
SPC = B // N_CORES           # samples per core = 4
NN = 4096                    # raw n positions (incl fakes/junk)
NCH = 32                     # chunks of 128
NT = 4                       # conv n-tiles of 1024
NVALID = 63 * 63             # real spatial positions
XW = H * W_SP + 128          # padded x row
XT = 1120                    # per-n-tile x slice (1024 + 96 window slack)

KREPS = int(os.environ.get("KREPS", "1"))
DBG = int(os.environ.get("DBG", "0"))
VARIANT = os.environ.get("VARIANT", "")  # "", "convonly", "nodr"

_BUILT = {}


def _build_nc():
    import concourse.bacc as bacc
    import concourse.mybir as mybir
    import concourse.tile as tile

    F32 = mybir.dt.float32
    BF16 = mybir.dt.bfloat16
    F8 = mybir.dt.float8e4

    nc = bacc.Bacc("TRN2")

    xs = nc.dram_tensor("xs", [SPC, 128, XW], BF16, kind="ExternalInput")
    wta = nc.dram_tensor("wta", [128, 4 * 8 * 128], BF16, kind="ExternalInput")
    # f32 consts: maskbd [128,256] | mask2 [128,32] | maskdiag8 [0:32,1024]
    #   | rm8 [128,32] | beta_col [128,8]
    cons_f = nc.dram_tensor("cons_f", [128, 256 + 32 + 1024 + 32 + 8],
                            F32, kind="ExternalInput")
    # bf16 consts: eye [128,128] | irep [0:32,128] | rhs5 [128,32]
    cons_h = nc.dram_tensor("cons_h", [128, 128 + 128 + 32], BF16,
                            kind="ExternalInput")
    out_d = nc.dram_tensor("out", [SPC, 32, 32], F32, kind="ExternalOutput")
    dbg = None
    if DBG:
        dbg = {
            "ua": nc.dram_tensor("dbg_ua", [128, 8, 128], BF16,
                                 kind="ExternalOutput"),
            "ut": nc.dram_tensor("dbg_ut", [128, 2, 1024], F8,
                                 kind="ExternalOutput"),
            "o0": nc.dram_tensor("dbg_o0", [32, 32], F32,
                                 kind="ExternalOutput"),
            "pb": nc.dram_tensor("dbg_pb", [32, 1024], F32,
                                 kind="ExternalOutput"),
            "e": nc.dram_tensor("dbg_e", [32, 1024], BF16,
                                kind="ExternalOutput"),
            "ct": nc.dram_tensor("dbg_ct", [128, 8, 32], F8,
                                 kind="ExternalOutput"),
            "po": nc.dram_tensor("dbg_po", [32, 1024], F32,
                                 kind="ExternalOutput"),
            "onx": nc.dram_tensor("dbg_onx", [32, 32], F32,
                                  kind="ExternalOutput"),
            "rn": nc.dram_tensor("dbg_rn", [32, 1], F32,
                                 kind="ExternalOutput"),
            "ob": nc.dram_tensor("dbg_ob", [128, 8, 32], BF16,
                                 kind="ExternalOutput"),
            "sa": nc.dram_tensor("dbg_sa", [128, 32], F32,
                                 kind="ExternalOutput"),
        }

    with nc.allow_low_precision("u stored bf16/fp8 by design"):
        with tile.TileContext(nc) as tc:
            _emit(nc, tc, mybir, F32, BF16, F8, xs, wta, cons_f, cons_h, out_d,
                  dbg)
    nc.compile()
    return nc


def _emit(nc, tc, mybir, F32, BF16, F8, xs, wta, cons_f, cons_h, out_d,
          dbg=None):
    from contextlib import ExitStack

    AF = mybir.ActivationFunctionType
    DR = mybir.MatmulPerfMode.DoubleRow
    AX = mybir.AxisListType.X
    OP = mybir.AluOpType

    with ExitStack() as ctx:
        const = ctx.enter_context(tc.tile_pool(name="const", bufs=1))
        big = ctx.enter_context(tc.tile_pool(name="big", bufs=1))
        xpool = ctx.enter_context(tc.tile_pool(name="xp", bufs=2))
        work = ctx.enter_context(tc.tile_pool(name="work", bufs=2))
        ps = ctx.enter_context(tc.tile_pool(name="ps", bufs=1, space="PSUM"))

        # pre-load the one ACT table set covering ln/exp/copy/identity/square
        # so the auto-placement pass never injects mid-kernel table swaps
        atl = mybir.InstLoadActFuncSet(name=nc.get_next_instruction_name(),
                                       ins=[], outs=[], act_func_set_id=6)
        nc.scalar.add_instruction(atl)

        wta_t = const.tile([128, 4, 8, 128], BF16)
        cf_t = const.tile([128, 256 + 32 + 1024 + 32 + 8], F32)
        ch_t = const.tile([128, 128 + 128 + 32], BF16)
        nc.gpsimd.dma_start(wta_t[:].rearrange("p a b c -> p (a b c)"),
                            wta[:, :])
        x0_t = xpool.tile([128, NT, XT], BF16)
        for nt in range(NT):
            nc.scalar.dma_start(x0_t[:, nt, :], xs[0, :, nt * 1024:nt * 1024 + XT])
        nc.gpsimd.dma_start(cf_t[:], cons_f[:, :])
        nc.gpsimd.dma_start(ch_t[:], cons_h[:, :])
        maskbd = cf_t[:, 0:256]
        mask2 = cf_t[:, 256:288]
        maskdiag8 = cf_t[0:32, 288:1312]
        rm8 = cf_t[:, 1312:1344]
        beta_col = cf_t[:, 1344:1352]
        eye = ch_t[:, 0:128]
        irep = ch_t[0:32, 128:256]
        rhs5 = ch_t[:, 256:288]

        UTDT = BF16 if VARIANT in ("nodr", "dmat") else F8
        ua = big.tile([128, 8, NN], BF16)          # u0+beta in [oc, n]
        ut = big.tile([128, NCH, OUT_C], UTDT)     # u0+beta in [n, oc]
        out_sb = big.tile([32, SPC, 32], F32)
        pending_t = []                             # chunks awaiting transpose

        def t_chunk(t):
            if VARIANT == "dmat":
                # ua -> ut via DMA crossbar transpose on the idle SP queue
                for blk in range(8):
                    nc.sync.dma_start_transpose(
                        ut[:, t, 128 * blk:128 * (blk + 1)],
                        ua[:, blk, 128 * t:128 * (t + 1)])
                return
            ptr = ps.tile([128, 8, 128], BF16, tag="pmix", bufs=2)
            for blk in range(8):
                nc.tensor.transpose(ptr[:, blk, :],
                                    ua[:, blk, 128 * t:128 * (t + 1)],
                                    eye[:, :])
            nc.vector.tensor_copy(ut[:, t, :],
                                  ptr[:].rearrange("p a b -> p (a b)"))

        def conv_phase(s, x_pre=None):
            """Conv tile-groups with transpose chunk-groups interleaved."""
            if x_pre is not None:
                x_t = x_pre
            else:
                x_t = xpool.tile([128, NT, XT], BF16)
                for nt in range(NT):
                    nc.gpsimd.dma_start(x_t[:, nt, :],
                                        xs[s, :, nt * 1024:nt * 1024 + XT])
            o0p = work.tile([128, 8, NT], F32, tag="o0p")

            for nt in range(NT):
                for blk in range(8):
                    pc = ps.tile([128, 1024], F32, tag="A", bufs=2)
                    for kpos in range(4):
                        kh, kw = kpos // 2, kpos % 2
                        off = kh * W_SP + kw
                        for h in range(2):
                            nc.tensor.matmul(
                                pc[:, 512 * h:512 * (h + 1)],
                                wta_t[:, kpos, blk, :],
                                x_t[:, nt, off + 512 * h: off + 512 * h + 512],
                                start=(kpos == 0), stop=(kpos == 3))
                    nc.scalar.activation(
                        ua[:, blk, nt * 1024:(nt + 1) * 1024], pc[:],
                        AF.Identity, bias=beta_col[:, blk:blk + 1],
                        accum_out=o0p[:, blk, nt:nt + 1])
                    # one transpose chunk per conv group keeps the DVE
                    # copies (1.2us each) fed without throttling the PE
                    if nt >= 1:
                        t_chunk(8 * (nt - 1) + blk)
            pending_t.extend(range(24, 32))
            return o0p

        def sum_sq(o_ap, tag):
            """ssq [32,1] = sum_j o^2 via ACT Square + accum."""
            sq = work.tile([32, 32], F32, tag=tag + "q")
            ss = work.tile([32, 1], F32, tag=tag)
            nc.scalar.activation(sq[:], o_ap, AF.Square, accum_out=ss[:])
            return ss

        o0p = conv_phase(0, x_pre=x0_t)

        if KREPS > 1:
            rep_ctx = tc.For_i(0, KREPS, 1)
            rep_ctx.__enter__()

        for s in range(SPC):
            if VARIANT == "convonly":
                while pending_t:
                    t_chunk(pending_t.pop(0))
                if s + 1 < SPC:
                    o0p = conv_phase(s + 1)
                continue
            if dbg is not None and s == 0:
                nc.gpsimd.dma_start(dbg["ua"][:, :, :], ua[:, :, 0:128])
                nc.gpsimd.dma_start(dbg["ut"][:, :, :], ut[:, 0:2, :])
            # ---- o0 = sum_valid (u0+beta) from conv-epilogue accum ----
            o0ch = work.tile([128, 8], F32, tag="o0ch")
            nc.vector.tensor_reduce(o0ch[:], o0p[:], axis=AX, op=OP.add)
            f1 = work.tile([128, 8], F32, tag="f1")
            nc.vector.tensor_reduce(f1[:], ua[:, :, 63:4032:64], axis=AX,
                                    op=OP.add)
            f2 = work.tile([128, 8], F32, tag="f2")
            nc.vector.tensor_reduce(f2[:], ua[:, :, 4032:4096], axis=AX,
                                    op=OP.add)
            nc.vector.tensor_sub(o0ch[:], o0ch[:], f1[:])
            nc.vector.tensor_sub(o0ch[:], o0ch[:], f2[:])
            lhsT5 = work.tile([128, 32], BF16, tag="lhsT5")
            nc.vector.tensor_tensor(
                lhsT5[:].rearrange("p (g q) -> p g q", q=4),
                o0ch[:].unsqueeze(2).broadcast_to([128, 8, 4]),
                mask2.rearrange("p (g q) -> p g q", q=4),
                op=OP.mult)
            po = ps.tile([32, 1024], F32, tag="po", bufs=1)
            nc.tensor.matmul(po[:, 0:32], lhsT5[:], rhs5, start=True,
                             stop=True)
            o0_sb = work.tile([32, 32], F32, tag="onx")
            nc.vector.tensor_copy(o0_sb[:], po[:, 0:32])
            o_cur = o0_sb
            if dbg is not None and s == 0:
                nc.gpsimd.dma_start(dbg["o0"][:, :], o0_sb[:])

            # ---- 2 routing iterations ----
            for it in range(2):
                # obdt is built from UNNORMALIZED o; the normalize scale
                # rn[i] = rsqrt(ssq) is folded into the exp's per-partition
                # scale, so the rsqrt (Ln/Exp, may swap ACT tables) runs off
                # the critical path while the b-mm streams.
                o_c16 = work.tile([32, 32], BF16, tag="oc16")
                nc.vector.tensor_copy(o_c16[:], o_cur[:])
                ssq = sum_sq(o_cur[:], "s1")
                lns = work.tile([32, 1], F32, tag="s2")
                nc.scalar.activation(lns[:], ssq[:], AF.Ln)
                rn = work.tile([32, 1], F32, tag="s3")
                nc.scalar.activation(rn[:], lns[:], AF.Exp, scale=-0.5)

                # obdt[oc, blk, i]: transpose o, strip-replicate, mask
                oTp = ps.tile([128, 8, 128], BF16, tag="pmix", bufs=2)
                nc.tensor.transpose(oTp[0:32, 0, 0:32], o_c16[:],
                                    eye[0:32, 0:32])
                oT_sb = work.tile([32, 32], BF16, tag="oT")
                nc.vector.tensor_copy(oT_sb[:], oTp[0:32, 0, 0:32])
                sA = ps.tile([128, 1024], F32, tag="A", bufs=2)
                nc.tensor.matmul(sA[:, 0:32], irep, oT_sb[:], start=True,
                                 stop=True)
                obdt = work.tile([128, 8, 32], BF16, tag="obdt")
                nc.vector.tensor_tensor(
                    obdt[:],
                    sA[:, 0:32].unsqueeze(1).broadcast_to([128, 8, 32]),
                    maskbd.rearrange("p (g i) -> p g i", i=32),
                    op=OP.mult)
                if dbg is not None and s == 0 and it == 0:
                    nc.gpsimd.dma_start(dbg["ob"][:, :, :], obdt[:])
                    sast = work.tile([128, 32], F32, tag="dbgsa")
                    nc.vector.tensor_copy(sast[:], sA[:, 0:32])
                    nc.gpsimd.dma_start(dbg["sa"][:, :], sast[:])

                # 4 passes of 1024 n: b-mm -> exp -> eT/softmax -> o-mm
                pbs = [None] * 4
                es = [None] * 4

                def b_pass(p):
                    # one 512-wide region per PSUM bank: start=True clears a
                    # whole bank's has_written, so exactly one start may land
                    # in each bank per accumulation group
                    pb = ps.tile([128, 1024], F32, tag="A", bufs=2)
                    for g in range(8):
                        for dc in range(2):
                            nc.tensor.matmul(
                                pb[0:32, 512 * dc:512 * (dc + 1)],
                                obdt[:, g, :],
                                ua[:, g, 1024 * p + 512 * dc:
                                   1024 * p + 512 * (dc + 1)],
                                start=(g == 0), stop=(g == 7))
                    pbs[p] = pb

                def exp_pass(p):
                    e_sb = work.tile([32, 1024], BF16, tag="esb")
                    nc.scalar.activation(e_sb[:], pbs[p][0:32, :], AF.Exp,
                                         scale=rn[:])
                    es[p] = e_sb

                cts = [None] * 4

                def eT_pass(p):
                    # transpose exp -> [n, i] and build ct = 8*e/z (fp8);
                    # the DVE chain runs while the PE streams other passes
                    eT = ps.tile([128, 8, 128], BF16, tag="pmix", bufs=2)
                    for c8 in range(8):
                        nc.tensor.transpose(eT[:, c8, 0:32],
                                            es[p][:, 128 * c8:128 * (c8 + 1)],
                                            eye[0:32, 0:32])
                    z = work.tile([128, 8], F32, tag="z")
                    nc.vector.tensor_reduce(z[:], eT[:, :, 0:32], axis=AX,
                                            op=OP.add)
                    zi = work.tile([128, 8], F32, tag="zi")
                    nc.vector.reciprocal(zi[:], z[:])
                    zi2 = work.tile([128, 8], F32, tag="zi2")
                    nc.vector.tensor_tensor(zi2[:], zi[:],
                                            rm8[:, 8 * p:8 * (p + 1)],
                                            op=OP.mult)
                    ct = work.tile([128, 8, 32], UTDT, tag="ct")
                    nc.vector.tensor_tensor(
                        ct[:], eT[:, :, 0:32],
                        zi2[:].unsqueeze(2).broadcast_to([128, 8, 32]),
                        op=OP.mult)
                    if dbg is not None and s == 0 and it == 0 and p == 0:
                        nc.gpsimd.dma_start(dbg["ct"][:, :, :], ct[:])
                    cts[p] = ct

                def omm_pass(p):
                    ct = cts[p]
                    if VARIANT in ("nodr", "dmat"):
                        for c in range(8):
                            t = 8 * p + c
                            for q in range(2):
                                nc.tensor.matmul(
                                    po[:, 512 * q:512 * (q + 1)],
                                    ct[:, c, :],
                                    ut[:, t, 512 * q:512 * (q + 1)],
                                    start=(t == 0), stop=(t == 31))
                        return
                    for dc in range(4):
                        t2 = 4 * p + dc
                        for q in range(2):
                            nc.tensor.matmul(
                                po[:, 512 * q:512 * (q + 1)],
                                ct[:, 2 * dc:2 * dc + 2, :],
                                ut[:, 2 * t2:2 * t2 + 2,
                                   512 * q:512 * (q + 1)],
                                start=(t2 == 0), stop=(t2 == 15),
                                perf_mode=DR)

                dbg_it = dbg is not None and s == 0 and it == 0
                b_pass(0)
                if dbg_it:
                    stg = work.tile([32, 1024], F32, tag="dbgs")
                    nc.vector.tensor_copy(stg[:], pbs[0][0:32, :])
                    nc.gpsimd.dma_start(dbg["pb"][:, :], stg[:])
                    nc.gpsimd.dma_start(dbg["rn"][:, :], rn[:])
                exp_pass(0)
                if dbg_it:
                    nc.gpsimd.dma_start(dbg["e"][:, :], es[0][:])
                while pending_t:
                    t_chunk(pending_t.pop(0))
                b_pass(1)
                eT_pass(0)
                exp_pass(1)
                b_pass(2)
                omm_pass(0)
                eT_pass(1)
                exp_pass(2)
                b_pass(3)
                omm_pass(1)
                eT_pass(2)
                exp_pass(3)
                omm_pass(2)
                eT_pass(3)
                omm_pass(3)
                if dbg_it:
                    stg2 = work.tile([32, 1024], F32, tag="dbgs")
                    nc.vector.tensor_copy(stg2[:], po[:])
                    nc.gpsimd.dma_start(dbg["po"][:, :], stg2[:])
                if it == 1 and s + 1 < SPC:
                    o0p = conv_phase(s + 1)  # overlap next conv with tail

                # diagonal extraction (maskdiag8 = 1/8 on diag blocks)
                tmpd = work.tile([32, 1024], F32, tag="tmpd")
                nc.vector.tensor_tensor(tmpd[:], po[:], maskdiag8,
                                        op=OP.mult)
                o_nx = work.tile([32, 32], F32, tag="onx")
                nc.vector.tensor_reduce(
                    o_nx[:], tmpd[:].rearrange("p (i k) -> p k i", k=32),
                    axis=AX, op=OP.add)
                if dbg is not None and s == 0 and it == 0:
                    nc.gpsimd.dma_start(dbg["onx"][:, :], o_nx[:])
                o_cur = o_nx

            # ---- squash ----
            ssq = sum_sq(o_cur[:], "s1")
            lns = work.tile([32, 1], F32, tag="s2")
            nc.scalar.activation(lns[:], ssq[:], AF.Ln)
            sq_s = work.tile([32, 1], F32, tag="s3")
            nc.scalar.activation(sq_s[:], lns[:], AF.Exp, scale=0.5)
            d2 = work.tile([32, 1], F32, tag="s4")
            nc.vector.tensor_scalar_add(d2[:], sq_s[:], 1e-6)
            r2 = work.tile([32, 1], F32, tag="s5")
            nc.vector.reciprocal(r2[:], d2[:])
            p1 = work.tile([32, 1], F32, tag="s6")
            nc.vector.tensor_scalar_add(p1[:], ssq[:], 1.0)
            r1 = work.tile([32, 1], F32, tag="s7")
            nc.vector.reciprocal(r1[:], p1[:])
            t1 = work.tile([32, 1], F32, tag="s8")
            nc.vector.tensor_tensor(t1[:], ssq[:], r1[:], op=OP.mult)
            f = work.tile([32, 1], F32, tag="s9")
            nc.vector.tensor_tensor(f[:], t1[:], r2[:], op=OP.mult)
            nc.scalar.activation(out_sb[:, s, :], o_cur[:], AF.Copy,
                                 scale=f[:])
            nc.gpsimd.dma_start(out_d[s, :, :], out_sb[:, s, :])

        if KREPS > 1:
            rep_ctx.__exit__(None, None, None)


def _consts():
    p = np.arange(128)
    i = np.arange(32)
    g = np.arange(8)
    maskbd = (i[None, None, :] == 4 * g[None, :, None] + p[:, None, None] // 32)
    mask2 = (p[:, None] // 32 == i[None, :] % 4)
    ch = np.arange(OUT_C)
    maskdiag = (ch[None, :] // 32 == i[:, None]) / 8.0
    maskdiag_p = np.zeros((128, OUT_C), np.float32)
    maskdiag_p[0:32] = maskdiag
    rm8 = np.full((128, 32), 8.0, np.float32)
    rm8[63, :] = 0.0
    rm8[127, :] = 0.0
    rm8[64:, 31] = 0.0
    return maskbd.reshape(128, 256).astype(np.float32), \
        mask2.astype(np.float32), maskdiag_p, rm8


def kernel(x, W, b_conv):
    from concourse.bass_utils import run_bass_kernel_spmd

    BF = ml_dtypes.bfloat16
    x = np.asarray(x, dtype=np.float32)
    W = np.asarray(W, dtype=np.float32)
    b_conv = np.asarray(b_conv, dtype=np.float32)

    # wtA[c, kpos, blk, j] = W[blk*128+j, c, kh, kw]
    wta = np.ascontiguousarray(
        W.reshape(8, 128, 128, 4).transpose(2, 3, 0, 1)
    ).reshape(128, 4 * 8 * 128).astype(BF)

    maskbd, mask2, maskdiag_p, rm8 = _consts()
    beta_col = np.ascontiguousarray(
        b_conv.reshape(8, 128).T).astype(np.float32)  # [p, blk]
    cons_f = np.concatenate(
        [maskbd, mask2, maskdiag_p, rm8, beta_col], axis=1).astype(np.float32)

    eye = np.eye(128, dtype=np.float32)
    irep_p = np.zeros((128, 128), np.float32)
    irep_p[0:32] = (np.arange(32)[:, None] == np.arange(128)[None, :] % 32)
    rhs5 = (np.arange(128)[:, None] % 32 == np.arange(32)[None, :])
    cons_h = np.concatenate(
        [eye, irep_p, rhs5.astype(np.float32)], axis=1).astype(BF)

    if "nc" not in _BUILT:
        _BUILT["nc"] = _build_nc()
    nc = _BUILT["nc"]

    xp = np.zeros((B, 128, XW), np.float32)
    xp[:, :, :H * W_SP] = x.reshape(B, 128, H * W_SP)
    xp = xp.astype(BF)

    in_maps = []
    for c in range(N_CORES):
        in_maps.append({"xs": np.ascontiguousarray(xp[c * SPC:(c + 1) * SPC]),
                        "wta": wta, "cons_f": cons_f, "cons_h": cons_h})

    global _last_in_maps
    _last_in_maps = in_maps
    res = run_bass_kernel_spmd(nc, in_maps, core_ids=list(range(N_CORES)))
    out = np.concatenate([r["out"] for r in res.results], axis=0)
    return out.astype(np.float32)


_last_in_maps = None


# revision 3
# speedup vs baseline: 1.0179x; 1.0179x over previous
"""Trainium2 Bass kernel v4 for nn_CapsuleLayer: 2x2 conv (128->1024) + routing.

Design (4 samples/core on 8 cores, data-parallel over batch):
  - Conv weights-stationary bf16: stationary wtA[c, kpos, blk, 128oc], moving
    x windows -> u_A[oc, n] in PSUM. Epilogue = ACT Identity copy with
    per-partition bias (conv bias folded here: ua = u0 + beta) and accum_out
    riding to produce o0 row-sums.
  - n = 4096 raw positions (32 chunks x 128), pos p = 64h + w; fakes at
    p%64==63, junk at p>=4032: zeroed via rm8 routing mask; o0 corrected by
    subtracting fake/junk column sums (f1/f2).
  - Transposes (PE, bf16) interleave with conv tile-groups so the DVE
    PSUM->SBUF copies (ptr -> ut fp8) hide under conv matmul streams.
  - b-mm flipped: obdt [128oc, 32i] bf16 stationary, streams ua ->
    pb [32i, 1024n] f32; ACT exp directly on PSUM (bias already in ua).
  - softmax: PE-transpose exp -> eT [n, i] bf16 PSUM, DVE z/recip; ct fp8
    scaled x8 (rm8 = 8, maskdiag = 1/8) to avoid fp8 subnormals.
  - o-mm: fp8 DoubleRow, ct2 [128, 2, 32] stationary, ut2 [128, 2, 256]
    moving, K=256 chunk pairs -> po [32 i, 1024 oc]; diag extraction on DVE.
  - PSUM: tagA [128,1024]f32 x2 (conv pc + pb + irep share), pmix
    [128,8,128]bf16 x2 (ptr + eT + oT), po [32,1024]f32 x1 = 8 banks.
"""
import os
import sys

sys.path.insert(0, "/opt/trn_rl_repo")

import numpy as np
import ml_dtypes

B, IN_C, H, W_SP = 32, 128, 64, 64
NUM_CAPS, D0, D1 = 32, 4, 8
OUT_C = NUM_CAPS * D0 * D1  # 1024
N_CORES = 8
SPC = B // N_CORES           # samples per core = 4
NN = 4096                    # raw n positions (incl fakes/junk)
NCH = 32                     # chunks of 128
NT = 4                       # conv n-tiles of 1024
NVALID = 63 * 63             # real spatial positions
XW = H * W_SP + 128          # padded x row
XT = 1120                    # per-n-tile x slice (1024 + 96 window slack)

KREPS = int(os.environ.get("KREPS", "1"))
DBG = int(os.environ.get("DBG", "0"))
VARIANT = os.environ.get("VARIANT", "")  # "", "convonly", "nodr"

_BUILT = {}


def _build_nc():
    import concourse.bacc as bacc
    import concourse.mybir as mybir
    import concourse.tile as tile

    F32 = mybir.dt.float32
    BF16 = mybir.dt.bfloat16
    F8 = mybir.dt.float8e4

    nc = bacc.Bacc("TRN2")

    xs = nc.dram_tensor("xs", [SPC, 128, XW], BF16, kind="ExternalInput")
    wta = nc.dram_tensor("wta", [128, 4 * 8 * 128], BF16, kind="ExternalInput")
    # f32 consts: maskbd [128,256] | mask2 [128,32] | maskdiag8 [0:32,1024]
    #   | rm8 [128,32] | beta_col [128,8]
    cons_f = nc.dram_tensor("cons_f", [128, 256 + 32 + 1024 + 32 + 8],
                            F32, kind="ExternalInput")
    # bf16 consts: eye [128,128] | irep [0:32,128] | rhs5 [128,32]
    cons_h = nc.dram_tensor("cons_h", [128, 128 + 128 + 32], BF16,
                            kind="ExternalInput")
    out_d = nc.dram_tensor("out", [SPC, 32, 32], F32, kind="ExternalOutput")
    dbg = None
    if DBG:
        dbg = {
            "ua": nc.dram_tensor("dbg_ua", [128, 8, 128], BF16,
                                 kind="ExternalOutput"),
            "ut": nc.dram_tensor("dbg_ut", [128, 2, 1024], F8,
                                 kind="ExternalOutput"),
            "o0": nc.dram_tensor("dbg_o0", [32, 32], F32,
                                 kind="ExternalOutput"),
            "pb": nc.dram_tensor("dbg_pb", [32, 1024], F32,
                                 kind="ExternalOutput"),
            "e": nc.dram_tensor("dbg_e", [32, 1024], BF16,
                                kind="ExternalOutput"),
            "ct": nc.dram_tensor("dbg_ct", [128, 8, 32], F8,
                                 kind="ExternalOutput"),
            "po": nc.dram_tensor("dbg_po", [32, 1024], F32,
                                 kind="ExternalOutput"),
            "onx": nc.dram_tensor("dbg_onx", [32, 32], F32,
                                  kind="ExternalOutput"),
            "rn": nc.dram_tensor("dbg_rn", [32, 1], F32,
                                 kind="ExternalOutput"),
            "ob": nc.dram_tensor("dbg_ob", [128, 8, 32], BF16,
                                 kind="ExternalOutput"),
            "sa": nc.dram_tensor("dbg_sa", [128, 32], F32,
                                 kind="ExternalOutput"),
        }

    with nc.allow_low_precision("u stored bf16/fp8 by design"):
        with tile.TileContext(nc) as tc:
            _emit(nc, tc, mybir, F32, BF16, F8, xs, wta, cons_f, cons_h, out_d,
                  dbg)
    nc.compile()
    return nc


def _emit(nc, tc, mybir, F32, BF16, F8, xs, wta, cons_f, cons_h, out_d,
          dbg=None):
    from contextlib import ExitStack

    AF = mybir.ActivationFunctionType
    DR = mybir.MatmulPerfMode.DoubleRow
    AX = mybir.AxisListType.X
    OP = mybir.AluOpType

    with ExitStack() as ctx:
        const = ctx.enter_context(tc.tile_pool(name="const", bufs=1))
        big = ctx.enter_context(tc.tile_pool(name="big", bufs=1))
        xpool = ctx.enter_context(tc.tile_pool(name="xp", bufs=2))
        work = ctx.enter_context(tc.tile_pool(name="work", bufs=2))
        ps = ctx.enter_context(tc.tile_pool(name="ps", bufs=1, space="PSUM"))

        # pre-load the one ACT table set covering ln/exp/copy/identity/square
        # so the auto-placement pass never injects mid-kernel table swaps
        atl = mybir.InstLoadActFuncSet(name=nc.get_next_instruction_name(),
                                       ins=[], outs=[], act_func_set_id=6)
        nc.scalar.add_instruction(atl)

        wta_t = const.tile([128, 8, 4, 128], BF16)
        cf_t = const.tile([128, 256 + 32 + 1024 + 32 + 8], F32)
        ch_t = const.tile([128, 128 + 128 + 32], BF16)
        wta_flat = wta_t[:].rearrange("p a b c -> p (a b c)")
        nc.gpsimd.dma_start(wta_flat[:, 0:512], wta[:, 0:512])
        nc.gpsimd.dma_start(wta_flat[:, 512:4096], wta[:, 512:4096])
        x0_t = xpool.tile([128, NT, XT], BF16)
        for nt in range(NT):
            nc.scalar.dma_start(x0_t[:, nt, :], xs[0, :, nt * 1024:nt * 1024 + XT])
        nc.gpsimd.dma_start(cf_t[:], cons_f[:, :])
        nc.gpsimd.dma_start(ch_t[:], cons_h[:, :])
        maskbd = cf_t[:, 0:256]
        mask2 = cf_t[:, 256:288]
        maskdiag8 = cf_t[0:32, 288:1312]
        rm8 = cf_t[:, 1312:1344]
        beta_col = cf_t[:, 1344:1352]
        eye = ch_t[:, 0:128]
        irep = ch_t[0:32, 128:256]
        rhs5 = ch_t[:, 256:288]

        UTDT = BF16 if VARIANT in ("nodr", "dmat") else F8
        ua = big.tile([128, 8, NN], BF16)          # u0+beta in [oc, n]
        ut = big.tile([128, NCH, OUT_C], UTDT)     # u0+beta in [n, oc]
        out_sb = big.tile([32, SPC, 32], F32)
        pending_t = []                             # chunks awaiting transpose

        def t_chunk(t):
            if VARIANT == "dmat":
                # ua -> ut via DMA crossbar transpose on the idle SP queue
                for blk in range(8):
                    nc.sync.dma_start_transpose(
                        ut[:, t, 128 * blk:128 * (blk + 1)],
                        ua[:, blk, 128 * t:128 * (t + 1)])
                return
            ptr = ps.tile([128, 8, 128], BF16, tag="pmix", bufs=2)
            for blk in range(8):
                nc.tensor.transpose(ptr[:, blk, :],
                                    ua[:, blk, 128 * t:128 * (t + 1)],
                                    eye[:, :])
            nc.vector.tensor_copy(ut[:, t, :],
                                  ptr[:].rearrange("p a b -> p (a b)"))

        def conv_phase(s, x_pre=None):
            """Conv tile-groups with transpose chunk-groups interleaved."""
            if x_pre is not None:
                x_t = x_pre
            else:
                x_t = xpool.tile([128, NT, XT], BF16)
                for nt in range(NT):
                    nc.gpsimd.dma_start(x_t[:, nt, :],
                                        xs[s, :, nt * 1024:nt * 1024 + XT])
            o0p = work.tile([128, 8, NT], F32, tag="o0p")

            for nt in range(NT):
                for blk in range(8):
                    pc = ps.tile([128, 1024], F32, tag="A", bufs=2)
                    for kpos in range(4):
                        kh, kw = kpos // 2, kpos % 2
                        off = kh * W_SP + kw
                        for h in range(2):
                            nc.tensor.matmul(
                                pc[:, 512 * h:512 * (h + 1)],
                                wta_t[:, blk, kpos, :],
                                x_t[:, nt, off + 512 * h: off + 512 * h + 512],
                                start=(kpos == 0), stop=(kpos == 3))
                    nc.scalar.activation(
                        ua[:, blk, nt * 1024:(nt + 1) * 1024], pc[:],
                        AF.Identity, bias=beta_col[:, blk:blk + 1],
                        accum_out=o0p[:, blk, nt:nt + 1])
                    # one transpose chunk per conv group keeps the DVE
                    # copies (1.2us each) fed without throttling the PE
                    if nt >= 1:
                        t_chunk(8 * (nt - 1) + blk)
            pending_t.extend(range(24, 32))
            return o0p

        def sum_sq(o_ap, tag):
            """ssq [32,1] = sum_j o^2 via ACT Square + accum."""
            sq = work.tile([32, 32], F32, tag=tag + "q")
            ss = work.tile([32, 1], F32, tag=tag)
            nc.scalar.activation(sq[:], o_ap, AF.Square, accum_out=ss[:])
            return ss

        o0p = conv_phase(0, x_pre=x0_t)

        if KREPS > 1:
            rep_ctx = tc.For_i(0, KREPS, 1)
            rep_ctx.__enter__()

        for s in range(SPC):
            if VARIANT == "convonly":
                while pending_t:
                    t_chunk(pending_t.pop(0))
                if s + 1 < SPC:
                    o0p = conv_phase(s + 1)
                continue
            if dbg is not None and s == 0:
                nc.gpsimd.dma_start(dbg["ua"][:, :, :], ua[:, :, 0:128])
                nc.gpsimd.dma_start(dbg["ut"][:, :, :], ut[:, 0:2, :])
            # ---- o0 = sum_valid (u0+beta) from conv-epilogue accum ----
            o0ch = work.tile([128, 8], F32, tag="o0ch")
            nc.vector.tensor_reduce(o0ch[:], o0p[:], axis=AX, op=OP.add)
            f1 = work.tile([128, 8], F32, tag="f1")
            nc.vector.tensor_reduce(f1[:], ua[:, :, 63:4032:64], axis=AX,
                                    op=OP.add)
            f2 = work.tile([128, 8], F32, tag="f2")
            nc.vector.tensor_reduce(f2[:], ua[:, :, 4032:4096], axis=AX,
                                    op=OP.add)
            nc.vector.tensor_sub(o0ch[:], o0ch[:], f1[:])
            nc.vector.tensor_sub(o0ch[:], o0ch[:], f2[:])
            lhsT5 = work.tile([128, 32], BF16, tag="lhsT5")
            nc.vector.tensor_tensor(
                lhsT5[:].rearrange("p (g q) -> p g q", q=4),
                o0ch[:].unsqueeze(2).broadcast_to([128, 8, 4]),
                mask2.rearrange("p (g q) -> p g q", q=4),
                op=OP.mult)
            po = ps.tile([32, 1024], F32, tag="po", bufs=1)
            nc.tensor.matmul(po[:, 0:32], lhsT5[:], rhs5, start=True,
                             stop=True)
            o0_sb = work.tile([32, 32], BF16, tag="onx")
            nc.vector.tensor_copy(o0_sb[:], po[:, 0:32])
            o_cur = o0_sb
            if dbg is not None and s == 0:
                nc.gpsimd.dma_start(dbg["o0"][:, :], o0_sb[:])

            # ---- 2 routing iterations ----
            for it in range(2):
                # obdt is built from UNNORMALIZED o; the normalize scale
                # rn[i] = rsqrt(ssq) is folded into the exp's per-partition
                # scale, so the rsqrt (Ln/Exp, may swap ACT tables) runs off
                # the critical path while the b-mm streams.
                ssq = sum_sq(o_cur[:], "s1")
                lns = work.tile([32, 1], F32, tag="s2")
                nc.scalar.activation(lns[:], ssq[:], AF.Ln)
                rn = work.tile([32, 1], F32, tag="s3")
                nc.scalar.activation(rn[:], lns[:], AF.Exp, scale=-0.5)

                # obdt[oc, blk, i]: transpose o, strip-replicate, mask
                oTp = ps.tile([128, 8, 128], BF16, tag="pmix", bufs=2)
                nc.tensor.transpose(oTp[0:32, 0, 0:32], o_cur[:],
                                    eye[0:32, 0:32])
                oT_sb = work.tile([32, 32], BF16, tag="oT")
                nc.vector.tensor_copy(oT_sb[:], oTp[0:32, 0, 0:32])
                sA = ps.tile([128, 1024], F32, tag="A", bufs=2)
                nc.tensor.matmul(sA[:, 0:32], irep, oT_sb[:], start=True,
                                 stop=True)
                obdt = work.tile([128, 8, 32], BF16, tag="obdt")
                nc.vector.tensor_tensor(
                    obdt[:],
                    sA[:, 0:32].unsqueeze(1).broadcast_to([128, 8, 32]),
                    maskbd.rearrange("p (g i) -> p g i", i=32),
                    op=OP.mult)
                if dbg is not None and s == 0 and it == 0:
                    nc.gpsimd.dma_start(dbg["ob"][:, :, :], obdt[:])
                    sast = work.tile([128, 32], F32, tag="dbgsa")
                    nc.vector.tensor_copy(sast[:], sA[:, 0:32])
                    nc.gpsimd.dma_start(dbg["sa"][:, :], sast[:])

                # 4 passes of 1024 n: b-mm -> exp -> eT/softmax -> o-mm
                pbs = [None] * 4
                es = [None] * 4

                def b_pass(p):
                    # one 512-wide region per PSUM bank: start=True clears a
                    # whole bank's has_written, so exactly one start may land
                    # in each bank per accumulation group
                    pb = ps.tile([128, 1024], F32, tag="A", bufs=2)
                    for g in range(8):
                        for dc in range(2):
                            nc.tensor.matmul(
                                pb[0:32, 512 * dc:512 * (dc + 1)],
                                obdt[:, g, :],
                                ua[:, g, 1024 * p + 512 * dc:
                                   1024 * p + 512 * (dc + 1)],
                                start=(g == 0), stop=(g == 7))
                    pbs[p] = pb

                def exp_pass(p):
                    e_sb = work.tile([32, 1024], BF16, tag="esb")
                    nc.scalar.activation(e_sb[:], pbs[p][0:32, :], AF.Exp,
                                         scale=rn[:])
                    es[p] = e_sb

                cts = [None] * 4

                def eT_pass(p):
                    # transpose exp -> [n, i] and build ct = 8*e/z (fp8);
                    # the DVE chain runs while the PE streams other passes
                    eT = ps.tile([128, 8, 128], BF16, tag="pmix", bufs=2)
                    for c8 in range(8):
                        nc.tensor.transpose(eT[:, c8, 0:32],
                                            es[p][:, 128 * c8:128 * (c8 + 1)],
                                            eye[0:32, 0:32])
                    z = work.tile([128, 8], F32, tag="z")
                    nc.vector.tensor_reduce(z[:], eT[:, :, 0:32], axis=AX,
                                            op=OP.add)
                    zi = work.tile([128, 8], F32, tag="zi")
                    nc.vector.reciprocal(zi[:], z[:])
                    zi2 = work.tile([128, 8], F32, tag="zi2")
                    nc.vector.tensor_tensor(zi2[:], zi[:],
                                            rm8[:, 8 * p:8 * (p + 1)],
                                            op=OP.mult)
                    ct = work.tile([128, 8, 32], UTDT, tag="ct")
                    nc.vector.tensor_tensor(
                        ct[:], eT[:, :, 0:32],
                        zi2[:].unsqueeze(2).broadcast_to([128, 8, 32]),
                        op=OP.mult)
                    if dbg is not None and s == 0 and it == 0 and p == 0:
                        nc.gpsimd.dma_start(dbg["ct"][:, :, :], ct[:])
                    cts[p] = ct

                def omm_pass(p):
                    ct = cts[p]
                    if VARIANT in ("nodr", "dmat"):
                        for c in range(8):
                            t = 8 * p + c
                            for q in range(2):
                                nc.tensor.matmul(
                                    po[:, 512 * q:512 * (q + 1)],
                                    ct[:, c, :],
                                    ut[:, t, 512 * q:512 * (q + 1)],
                                    start=(t == 0), stop=(t == 31))
                        return
                    for dc in range(4):
                        t2 = 4 * p + dc
                        for q in range(2):
                            nc.tensor.matmul(
                                po[:, 512 * q:512 * (q + 1)],
                                ct[:, 2 * dc:2 * dc + 2, :],
                                ut[:, 2 * t2:2 * t2 + 2,
                                   512 * q:512 * (q + 1)],
                                start=(t2 == 0), stop=(t2 == 15),
                                perf_mode=DR)

                dbg_it = dbg is not None and s == 0 and it == 0
                b_pass(0)
                if dbg_it:
                    stg = work.tile([32, 1024], F32, tag="dbgs")
                    nc.vector.tensor_copy(stg[:], pbs[0][0:32, :])
                    nc.gpsimd.dma_start(dbg["pb"][:, :], stg[:])
                    nc.gpsimd.dma_start(dbg["rn"][:, :], rn[:])
                exp_pass(0)
                if dbg_it:
                    nc.gpsimd.dma_start(dbg["e"][:, :], es[0][:])
                while pending_t:
                    t_chunk(pending_t.pop(0))
                b_pass(1)
                eT_pass(0)
                exp_pass(1)
                b_pass(2)
                omm_pass(0)
                eT_pass(1)
                exp_pass(2)
                b_pass(3)
                omm_pass(1)
                eT_pass(2)
                exp_pass(3)
                omm_pass(2)
                eT_pass(3)
                omm_pass(3)
                if dbg_it:
                    stg2 = work.tile([32, 1024], F32, tag="dbgs")
                    nc.vector.tensor_copy(stg2[:], po[:])
                    nc.gpsimd.dma_start(dbg["po"][:, :], stg2[:])
                if it == 1 and s + 1 < SPC:
                    o0p = conv_phase(s + 1)  # overlap next conv with tail

                # diagonal extraction (maskdiag8 = 1/8 on diag blocks)
                tmpd = work.tile([32, 1024], F32, tag="tmpd")
                nc.vector.tensor_tensor(tmpd[:], po[:], maskdiag8,
                                        op=OP.mult)
                o_nx = work.tile([32, 32], BF16, tag="onx")
                nc.vector.tensor_reduce(
                    o_nx[:], tmpd[:].rearrange("p (i k) -> p k i", k=32),
                    axis=AX, op=OP.add)
                if dbg is not None and s == 0 and it == 0:
                    nc.gpsimd.dma_start(dbg["onx"][:, :], o_nx[:])
                o_cur = o_nx

            # ---- squash ----
            ssq = sum_sq(o_cur[:], "s1")
            lns = work.tile([32, 1], F32, tag="s2")
            nc.scalar.activation(lns[:], ssq[:], AF.Ln)
            sq_s = work.tile([32, 1], F32, tag="s3")
            nc.scalar.activation(sq_s[:], lns[:], AF.Exp, scale=0.5)
            d2 = work.tile([32, 1], F32, tag="s4")
            nc.vector.tensor_scalar_add(d2[:], sq_s[:], 1e-6)
            r2 = work.tile([32, 1], F32, tag="s5")
            nc.vector.reciprocal(r2[:], d2[:])
            p1 = work.tile([32, 1], F32, tag="s6")
            nc.vector.tensor_scalar_add(p1[:], ssq[:], 1.0)
            r1 = work.tile([32, 1], F32, tag="s7")
            nc.vector.reciprocal(r1[:], p1[:])
            t1 = work.tile([32, 1], F32, tag="s8")
            nc.vector.tensor_tensor(t1[:], ssq[:], r1[:], op=OP.mult)
            f = work.tile([32, 1], F32, tag="s9")
            nc.vector.tensor_tensor(f[:], t1[:], r2[:], op=OP.mult)
            nc.scalar.activation(out_sb[:, s, :], o_cur[:], AF.Copy,
                                 scale=f[:])
            nc.gpsimd.dma_start(out_d[s, :, :], out_sb[:, s, :])

        if KREPS > 1:
            rep_ctx.__exit__(None, None, None)


def _consts():
    p = np.arange(128)
    i = np.arange(32)
    g = np.arange(8)
    maskbd = (i[None, None, :] == 4 * g[None, :, None] + p[:, None, None] // 32)
    mask2 = (p[:, None] // 32 == i[None, :] % 4)
    ch = np.arange(OUT_C)
    maskdiag = (ch[None, :] // 32 == i[:, None]) / 8.0
    maskdiag_p = np.zeros((128, OUT_C), np.float32)
    maskdiag_p[0:32] = maskdiag
    rm8 = np.full((128, 32), 8.0, np.float32)
    rm8[63, :] = 0.0
    rm8[127, :] = 0.0
    rm8[64:, 31] = 0.0
    return maskbd.reshape(128, 256).astype(np.float32), \
        mask2.astype(np.float32), maskdiag_p, rm8


def kernel(x, W, b_conv):
    from concourse.bass_utils import run_bass_kernel_spmd

    BF = ml_dtypes.bfloat16
    x = np.asarray(x, dtype=np.float32)
    W = np.asarray(W, dtype=np.float32)
    b_conv = np.asarray(b_conv, dtype=np.float32)

    # wtA[c, kpos, blk, j] = W[blk*128+j, c, kh, kw]
    wta = np.ascontiguousarray(
        W.reshape(8, 128, 128, 4).transpose(2, 0, 3, 1)  # [c, blk, kpos, j]
    ).reshape(128, 4 * 8 * 128).astype(BF)

    maskbd, mask2, maskdiag_p, rm8 = _consts()
    beta_col = np.ascontiguousarray(
        b_conv.reshape(8, 128).T).astype(np.float32)  # [p, blk]
    cons_f = np.concatenate(
        [maskbd, mask2, maskdiag_p, rm8, beta_col], axis=1).astype(np.float32)

    eye = np.eye(128, dtype=np.float32)
    irep_p = np.zeros((128, 128), np.float32)
    irep_p[0:32] = (np.arange(32)[:, None] == np.arange(128)[None, :] % 32)
    rhs5 = (np.arange(128)[:, None] % 32 == np.arange(32)[None, :])
    cons_h = np.concatenate(
        [eye, irep_p, rhs5.astype(np.float32)], axis=1).astype(BF)

    if "nc" not in _BUILT:
        _BUILT["nc"] = _build_nc()
    nc = _BUILT["nc"]

    xp = np.zeros((B, 128, XW), np.float32)
    xp[:, :, :H * W_SP] = x.reshape(B, 128, H * W_SP)
    xp = xp.astype(BF)

    in_maps = []
    for c in range(N_CORES):
        in_maps.append({"xs": np.ascontiguousarray(xp[c * SPC:(c + 1) * SPC]),
                        "wta": wta, "cons_f": cons_f, "cons_h": cons_h})

    global _last_in_maps
    _last_in_maps = in_maps
    res = run_bass_kernel_spmd(nc, in_maps, core_ids=list(range(N_CORES)))
    out = np.concatenate([r["out"] for r in res.results], axis=0)
    return out.astype(np.float32)


_last_in_maps = None
